# revision 1
# baseline (speedup 1.0000x reference)
# Malvar demosaic on 8 Trainium2 NeuronCores — pure data parallel (1 batch
# image per core).
#
# Strategy: polyphase decomposition. Each output (channel, Bayer-parity)
# plane at quarter resolution is a short sum of terms
#   (input phase, horizontal phase-shift) x (vertical 3-tap band),
# computed as banded [128 x 126] fp32r matmuls on the TensorEngine (vertical
# mixing across partitions) with horizontal shifts expressed as strided rhs
# column reads. Reflection padding is folded into the band matrices of the
# first/last row blocks; the 4 passthrough planes route through PE as
# identity bands (PE is the only engine that can shift across partitions).
# DVE clips conv results to [0,1] while interleaving even/odd output columns
# into assembled full-resolution row tiles; ScalarE copies passthroughs.
# All HBM traffic is contiguous >= 8KB per partition line.
import numpy as np
from contextlib import ExitStack


# ---------------------------------------------------------------------------
# Problem constants (hardcoded per harness contract)
B, H, W = 8, 2048, 2048
N_CORES = 8


def MALVAR_KERNELS():
    g = np.array([[0, 0, -1, 0, 0], [0, 0, 2, 0, 0], [-1, 2, 4, 2, -1],
                  [0, 0, 2, 0, 0], [0, 0, -1, 0, 0]], np.float32) / 8.0
    col = np.array([[0, 0, 0.5, 0, 0], [0, -1, 0, -1, 0], [-1, 4, 5, 4, -1],
                    [0, -1, 0, -1, 0], [0, 0, 0.5, 0, 0]], np.float32) / 8.0
    row = np.array([[0, 0, -1, 0, 0], [0, -1, 4, -1, 0], [0.5, 0, 5, 0, 0.5],
                    [0, -1, 4, -1, 0], [0, 0, -1, 0, 0]], np.float32) / 8.0
    br = np.array([[0, 0, -1.5, 0, 0], [0, 2, 0, 2, 0], [-1.5, 0, 6, 0, -1.5],
                   [0, 2, 0, 2, 0], [0, 0, -1.5, 0, 0]], np.float32) / 8.0
    return {"g": g, "col": col, "row": row, "br": br}


# (out channel, row parity di0, col parity dj0, kernel name)
CONV_OUTPUTS = [
    (1, 0, 0, "g"),    # green at R
    (2, 0, 0, "br"),   # blue  at R
    (0, 0, 1, "col"),  # red   at Gr
    (2, 0, 1, "row"),  # blue  at Gr
    (0, 1, 0, "row"),  # red   at Gb
    (2, 1, 0, "col"),  # blue  at Gb
    (0, 1, 1, "br"),   # red   at B
    (1, 1, 1, "g"),    # green at B
]
# passthrough planes: out[ch, 2i+di0, 2j+dj0] = x[2i+di0, 2j+dj0]
PASSTHROUGH_OUTPUTS = [(0, 0, 0), (1, 0, 1), (1, 1, 0), (2, 1, 1)]


def gen_passes(kernels=None):
    """Polyphase decomposition of each output plane.

    Returns a list of 12 dicts {ch, di0, dj0, is_pass, passes} where passes
    is a list of {pr, pc, dcol, taps: {drow: coeff}}. Output plane value:
      out[i, j] = sum over passes, taps:
          coeff * phase[pr,pc][i + drow, j + dcol]
    for output full-res site (2i + di0, 2j + dj0).
    """
    if kernels is None:
        kernels = MALVAR_KERNELS()
    qs = []
    for ch, di0, dj0, kname in CONV_OUTPUTS:
        k = kernels[kname]
        groups = {}
        for u in range(-2, 3):
            for v in range(-2, 3):
                c = float(k[u + 2, v + 2])
                if c == 0.0:
                    continue
                pr = (di0 + u) % 2
                drow = (di0 + u - pr) // 2
                pc = (dj0 + v) % 2
                dcol = (dj0 + v - pc) // 2
                key = (pr, pc, dcol)
                groups.setdefault(key, {})
                groups[key][drow] = groups[key].get(drow, 0.0) + c
        passes = [{"pr": pr, "pc": pc, "dcol": dcol, "taps": taps}
                  for (pr, pc, dcol), taps in sorted(groups.items())]
        qs.append({"ch": ch, "di0": di0, "dj0": dj0, "is_pass": False,
                   "passes": passes})
    for ch, di0, dj0 in PASSTHROUGH_OUTPUTS:
        qs.append({"ch": ch, "di0": di0, "dj0": dj0, "is_pass": True,
                   "passes": [{"pr": di0, "pc": dj0, "dcol": 0,
                               "taps": {0: 1.0}}]})
    return qs


def block_plan(n):
    """Row-block plan over n phase rows. Returns [(base, out0, M, cls)].

    Block covers output phase rows [out0, out0+M); its input tiles hold
    phase rows [base, base+128). cls: 0 first (reflect top), 1 interior,
    2 last (reflect bottom).
    """
    assert n >= 128
    plan = []
    out0 = 0
    while out0 < n:
        if out0 == 0:
            base, cls, M = 0, 0, 126
        elif out0 <= n - 127:
            base, cls, M = out0 - 1, 1, 126
        else:
            base, cls, M = n - 128, 2, n - out0
        plan.append((base, out0, M, cls))
        out0 += M
    return plan


def _class_geometry(n, cls):
    plan = block_plan(n)
    if cls == 0:
        return plan[0]
    if cls == 2:
        return plan[-1]
    interior = [b for b in plan if b[3] == 1]
    return interior[0] if interior else None


def gen_bands(n, cls, kernels=None):
    """Band (lhsT) matrices [128, 126] for every (q, pass) for block class
    cls. lhsT[k, m] = coeff so that psum[m, :] += sum_k lhsT[k, m]*tile[k, :]
    computes output phase row out0+m from tile rows (phase rows base+k),
    with reflection rows folded in."""
    qs = gen_passes(kernels)
    geo = _class_geometry(n, cls)
    bands = {}
    for qi, q in enumerate(qs):
        for pi, p in enumerate(q["passes"]):
            B = np.zeros((128, 126), np.float32)
            if geo is not None:
                base, out0, M, _ = geo
                pr = p["pr"]
                for m in range(126):
                    if out0 + m >= n:
                        continue
                    for drow, coeff in p["taps"].items():
                        r = out0 + m + drow
                        if r < 0:
                            r = -r - pr          # reflect top (same parity)
                        elif r >= n:
                            r = 2 * n - 1 - r - pr  # reflect bottom
                        k = r - base
                        assert 0 <= k < 128, (cls, qi, pi, m, drow, k)
                        B[k, m] += coeff
            bands[(qi, pi)] = B
    return bands


def build_bands_np(n, kernels=None):
    """[3, 128, NPT*126] f32 band tensor (partition-major for fast DMA)."""
    qs = gen_passes(kernels)
    npt = sum(len(q["passes"]) for q in qs)
    arr = np.zeros((3, 128, npt * 126), np.float32)
    for cls in range(3):
        bands = gen_bands(n, cls, kernels)
        g = 0
        for qi, q in enumerate(qs):
            for pi in range(len(q["passes"])):
                arr[cls, :, g * 126:(g + 1) * 126] = bands[(qi, pi)]
                g += 1
    return np.ascontiguousarray(arr)


# ---------------------------------------------------------------------------
# Bass module
def build_nc(H_, W_, kernels=None, num_devices=N_CORES, repeat=1,
             ablate=frozenset(), in_bufs=2, out_bufs=2, band_bufs=2,
             psum_bufs=8, split_a=False, act_clips=0):
    import concourse.bacc as bacc
    import concourse.tile as tile
    import concourse.mybir as mybir

    F32 = mybir.dt.float32
    F32R = mybir.dt.float32r

    n, wn = H_ // 2, W_ // 2
    NCH = min(512, wn)           # matmul moving free dim (one PSUM bank fp32)
    assert wn % NCH == 0
    nchunks = wn // NCH
    qs = gen_passes(kernels)
    gpi_of = {}
    g = 0
    for qi, q in enumerate(qs):
        for pi in range(len(q["passes"])):
            gpi_of[(qi, pi)] = g
            g += 1
    NPT = g
    plan = block_plan(n)

    nc = bacc.Bacc("TRN2", target_bir_lowering=False, debug=False,
                   enable_asserts=False, num_devices=num_devices)
    # float32r end-to-end on the matmul input path: the PE consumes fp32r
    # (rounded fp32) at full rate; the verifier requires producers typed f32r.
    x = nc.dram_tensor("x", [H_, W_], F32R, kind="ExternalInput").ap()
    bands_d = nc.dram_tensor("bands", [3, 128, NPT * 126], F32R,
                             kind="ExternalInput").ap()
    y = nc.dram_tensor("y", [3, H_, W_], F32, kind="ExternalOutput").ap()

    with ExitStack() as ctx:
        tc = ctx.enter_context(tile.TileContext(nc))
        in_pool = ctx.enter_context(tc.tile_pool(name="inp", bufs=in_bufs))
        band_pool = ctx.enter_context(tc.tile_pool(name="band", bufs=band_bufs))
        out_pool = ctx.enter_context(tc.tile_pool(name="outp", bufs=out_bufs))
        psum_pool = ctx.enter_context(tc.tile_pool(name="ps", bufs=psum_bufs,
                                                   space="PSUM"))
        if repeat > 1:
            # timing mode: run the whole body `repeat` times in one NEFF so
            # per-iteration HW time can be extracted from wall-clock deltas
            loop_cm = tc.For_i(0, repeat, 1)
            ctx.enter_context(loop_cm)

        band_tiles = {}

        def get_band_tile(cls):
            if cls not in band_tiles:
                bt = band_pool.tile([128, NPT * 126], F32R, tag="bands")
                nc.sync.dma_start(bt[:, :], bands_d[cls])
                band_tiles[cls] = bt
            return band_tiles[cls]

        for (base, out0, M, cls) in plan:
            bt = get_band_tile(cls)
            tin = {}
            for pr in (0, 1):
                t = in_pool.tile([128, W_ + 4], F32R, tag=f"t{pr}")
                if "dmain" not in ablate:
                    nc.sync.dma_start(t[:, 2:W_ + 2],
                                      x[2 * base + pr: 2 * base + pr + 255: 2, :])
                if "pad" not in ablate:
                    # reflect-pad columns: tile col c <-> image col c-2
                    nc.scalar.copy(t[:, 0:1], t[:, 4:5])
                    nc.scalar.copy(t[:, 1:2], t[:, 3:4])
                    nc.scalar.copy(t[:, W_ + 2:W_ + 3], t[:, W_:W_ + 1])
                    nc.scalar.copy(t[:, W_ + 3:W_ + 4], t[:, W_ - 1:W_])
                tin[pr] = t
            if split_a:
                A = {(ch, dy, hf): out_pool.tile([128, W_ // nchunks], F32,
                                                 tag=f"A{ch}{dy}h{hf}",
                                                 name=f"A{ch}{dy}h{hf}")
                     for ch in range(3) for dy in (0, 1)
                     for hf in range(nchunks)}
            else:
                A = {(ch, dy): out_pool.tile([128, W_], F32, tag=f"A{ch}{dy}",
                                             name=f"A{ch}{dy}")
                     for ch in range(3) for dy in (0, 1)}
            for qi, q in enumerate(qs):
                ch, di0, dj0 = q["ch"], q["di0"], q["dj0"]
                npass = len(q["passes"])
                for c in range(nchunks):
                    ps = psum_pool.tile([128, NCH], F32, tag="ps")
                    if "pe" not in ablate:
                        plist = q["passes"][:1] if "pe1" in ablate else q["passes"]
                        for pi, p in enumerate(plist):
                            gp = gpi_of[(qi, pi)]
                            lhsT = bt[:, gp * 126: gp * 126 + 126]
                            c0 = 2 * p["dcol"] + p["pc"] + 2 + 2 * NCH * c
                            rhs = tin[p["pr"]][:, c0: c0 + 2 * NCH - 1: 2]
                            nc.tensor.matmul(ps[0:126, :], lhsT, rhs,
                                             start=(pi == 0),
                                             stop=(pi == len(plist) - 1))
                    if "evac" in ablate:
                        continue
                    if split_a:
                        dest = A[(ch, di0, c)][0:126,
                                               dj0: 2 * NCH + dj0 - 1: 2]
                    else:
                        dest = A[(ch, di0)][0:126,
                                            2 * NCH * c + dj0:
                                            2 * NCH * (c + 1) + dj0 - 1: 2]
                    if q["is_pass"]:
                        nc.scalar.copy(dest, ps[0:126, :])
                    else:
                        nc.vector.tensor_scalar(
                            dest, ps[0:126, :], 1.0, 0.0,
                            mybir.AluOpType.min, mybir.AluOpType.max)
            if "dmaout" not in ablate:
                if split_a:
                    wh = W_ // nchunks
                    for (ch, dy, hf), t in A.items():
                        nc.sync.dma_start(
                            y[ch, 2 * out0 + dy: 2 * out0 + dy + 2 * M - 1: 2,
                              wh * hf: wh * (hf + 1)],
                            t[0:M, :])
                else:
                    for (ch, dy), t in A.items():
                        nc.sync.dma_start(
                            y[ch,
                              2 * out0 + dy: 2 * out0 + dy + 2 * M - 1: 2, :],
                            t[0:M, :])
    nc.compile()
    return nc


# ---------------------------------------------------------------------------
_NC_CACHE = {}


_LAST_RESULTS = None


def kernel(**inputs) -> np.ndarray:
    import os
    from concourse import bass_utils

    bayer = np.asarray(inputs["bayer"], dtype=np.float32)
    b, c1, h, w = bayer.shape
    assert (b, c1, h, w) == (B, 1, H, W), bayer.shape

    kernels = None
    if "k_g_at_rb" in inputs:
        kernels = {
            "g": np.asarray(inputs["k_g_at_rb"], np.float32).reshape(5, 5),
            "col": np.asarray(inputs["k_rb_at_g_col"], np.float32).reshape(5, 5),
            "row": np.asarray(inputs["k_rb_at_g_row"], np.float32).reshape(5, 5),
            "br": np.asarray(inputs["k_rb_at_br"], np.float32).reshape(5, 5),
        }

    repeat = int(os.environ.get("DEMOSAIC_REPEAT", "1"))
    key = (h, w, repeat)
    if key not in _NC_CACHE:
        import time as _time
        _t0 = _time.time()
        _NC_CACHE[key] = build_nc(h, w, kernels, repeat=repeat)
        if repeat > 1:
            print(f"[kernel] built module repeat={repeat} "
                  f"in {_time.time()-_t0:.1f}s", flush=True)
    nc = _NC_CACHE[key]

    bands_np = build_bands_np(h // 2, kernels)
    in_maps = [{"x": np.ascontiguousarray(bayer[i, 0]), "bands": bands_np}
               for i in range(N_CORES)]
    trace = os.environ.get("DEMOSAIC_TRACE", "0") == "1"
    res = bass_utils.run_bass_kernel_spmd(nc, in_maps,
                                          core_ids=list(range(N_CORES)),
                                          trace=trace)
    global _LAST_RESULTS
    _LAST_RESULTS = res
    out = np.stack([r["y"] for r in res.results], axis=0)
    return out.astype(np.float32, copy=False)


if __name__ == "__main__":
    # smoke: band/pass structure
    qs = gen_passes()
    for q in qs:
        print(q["ch"], q["di0"], q["dj0"], "passes:", len(q["passes"]),
              "pass" if q["is_pass"] else "conv")
    print("total passes:", sum(len(q["passes"]) for q in qs))
    print("plan n=1024:", block_plan(1024))



# revision 2
# speedup vs baseline: 6.4792x; 6.4792x over previous
# Malvar demosaic on 8 Trainium2 NeuronCores — pure data parallel (1 batch
# image per core).
#
# The deployment target is axon-tunneled NeuronCores, where host<->device
# bandwidth (~40 MB/s each way over the tunnel) dominates end-to-end time,
# so the design minimizes tunnel bytes:
#   - input ships as uint8 (bayer * 255 rounded): 32 MiB instead of 128.
#   - the device returns only the 8 interpolated (channel, Bayer-parity)
#     quarter-res planes as uint8 (64 MiB); the 4 passthrough planes are
#     filled host-side from the original fp32 input (exact).
#   - output staging buffers are created on-device (jnp.zeros) instead of
#     uploading host zeros; band matrices are cached on-device across calls.
# Device compute is exact up to input quantization: u8 pixel values (0..255)
# and the 1/16-multiple Malvar coefficients are exactly representable in
# bf16, products accumulate in fp32 PSUM, and the DVE's f32->u8 store
# rounds to nearest. Worst-case |err| <= 2.5*0.5/255 + 0.5/255 ~= 6.9e-3.
#
# Kernel strategy: polyphase decomposition. Each output (channel, parity)
# plane at quarter resolution is a short sum of terms
#   (input phase, horizontal phase-shift) x (vertical 3-tap band),
# computed as banded [128 x 126] bf16 matmuls on the TensorEngine (vertical
# mixing across partitions) with horizontal shifts expressed as strided rhs
# column reads. Reflection padding is folded into the band matrices of the
# first/last row blocks; column reflection is 4 ScalarE copies per tile.
import numpy as np
from contextlib import ExitStack


# ---------------------------------------------------------------------------
# Problem constants (hardcoded per harness contract)
B, H, W = 8, 2048, 2048
N_CORES = 8


def MALVAR_KERNELS():
    g = np.array([[0, 0, -1, 0, 0], [0, 0, 2, 0, 0], [-1, 2, 4, 2, -1],
                  [0, 0, 2, 0, 0], [0, 0, -1, 0, 0]], np.float32) / 8.0
    col = np.array([[0, 0, 0.5, 0, 0], [0, -1, 0, -1, 0], [-1, 4, 5, 4, -1],
                    [0, -1, 0, -1, 0], [0, 0, 0.5, 0, 0]], np.float32) / 8.0
    row = np.array([[0, 0, -1, 0, 0], [0, -1, 4, -1, 0], [0.5, 0, 5, 0, 0.5],
                    [0, -1, 4, -1, 0], [0, 0, -1, 0, 0]], np.float32) / 8.0
    br = np.array([[0, 0, -1.5, 0, 0], [0, 2, 0, 2, 0], [-1.5, 0, 6, 0, -1.5],
                   [0, 2, 0, 2, 0], [0, 0, -1.5, 0, 0]], np.float32) / 8.0
    return {"g": g, "col": col, "row": row, "br": br}


# (out channel, row parity di0, col parity dj0, kernel name)
CONV_OUTPUTS = [
    (1, 0, 0, "g"),    # green at R
    (2, 0, 0, "br"),   # blue  at R
    (0, 0, 1, "col"),  # red   at Gr
    (2, 0, 1, "row"),  # blue  at Gr
    (0, 1, 0, "row"),  # red   at Gb
    (2, 1, 0, "col"),  # blue  at Gb
    (0, 1, 1, "br"),   # red   at B
    (1, 1, 1, "g"),    # green at B
]
# passthrough planes (host-side): out[ch, 2i+di0, 2j+dj0] = x[2i+di0, 2j+dj0]
PASSTHROUGH_OUTPUTS = [(0, 0, 0), (1, 0, 1), (1, 1, 0), (2, 1, 1)]


def gen_passes(kernels=None):
    """Polyphase decomposition of each interpolated output plane.

    Returns a list of 8 dicts {ch, di0, dj0, passes} where passes is a list
    of {pr, pc, dcol, taps: {drow: coeff}}. Output plane value:
      out[i, j] = sum over passes, taps:
          coeff * phase[pr,pc][i + drow, j + dcol]
    for output full-res site (2i + di0, 2j + dj0).
    """
    if kernels is None:
        kernels = MALVAR_KERNELS()
    qs = []
    for ch, di0, dj0, kname in CONV_OUTPUTS:
        k = kernels[kname]
        groups = {}
        for u in range(-2, 3):
            for v in range(-2, 3):
                c = float(k[u + 2, v + 2])
                if c == 0.0:
                    continue
                pr = (di0 + u) % 2
                drow = (di0 + u - pr) // 2
                pc = (dj0 + v) % 2
                dcol = (dj0 + v - pc) // 2
                key = (pr, pc, dcol)
                groups.setdefault(key, {})
                groups[key][drow] = groups[key].get(drow, 0.0) + c
        passes = [{"pr": pr, "pc": pc, "dcol": dcol, "taps": taps}
                  for (pr, pc, dcol), taps in sorted(groups.items())]
        qs.append({"ch": ch, "di0": di0, "dj0": dj0, "passes": passes})
    return qs


def block_plan(n):
    """Row-block plan over n phase rows. Returns [(base, out0, M, cls)].

    Block covers output phase rows [out0, out0+M); its input tiles hold
    phase rows [base, base+128). cls: 0 first (reflect top), 1 interior,
    2 last (reflect bottom).
    """
    assert n >= 128
    plan = []
    out0 = 0
    while out0 < n:
        if out0 == 0:
            base, cls, M = 0, 0, 126
        elif out0 <= n - 127:
            base, cls, M = out0 - 1, 1, 126
        else:
            base, cls, M = n - 128, 2, n - out0
        plan.append((base, out0, M, cls))
        out0 += M
    return plan


def _class_geometry(n, cls):
    plan = block_plan(n)
    if cls == 0:
        return plan[0]
    if cls == 2:
        return plan[-1]
    interior = [b for b in plan if b[3] == 1]
    return interior[0] if interior else None


def gen_bands(n, cls, kernels=None):
    """Band (lhsT) matrices [128, 126] for every (q, pass) for block class
    cls. lhsT[k, m] = coeff so that psum[m, :] += sum_k lhsT[k, m]*tile[k, :]
    computes output phase row out0+m from tile rows (phase rows base+k),
    with reflection rows folded in."""
    qs = gen_passes(kernels)
    geo = _class_geometry(n, cls)
    bands = {}
    for qi, q in enumerate(qs):
        for pi, p in enumerate(q["passes"]):
            Bm = np.zeros((128, 126), np.float32)
            if geo is not None:
                base, out0, M, _ = geo
                pr = p["pr"]
                for m in range(126):
                    if out0 + m >= n:
                        continue
                    for drow, coeff in p["taps"].items():
                        r = out0 + m + drow
                        if r < 0:
                            r = -r - pr          # reflect top (same parity)
                        elif r >= n:
                            r = 2 * n - 1 - r - pr  # reflect bottom
                        k = r - base
                        assert 0 <= k < 128, (cls, qi, pi, m, drow, k)
                        Bm[k, m] += coeff
            bands[(qi, pi)] = Bm
    return bands


def build_bands_np(n, kernels=None):
    """[3, 128, NPT*126] bf16 band tensor (partition-major for fast DMA)."""
    import ml_dtypes
    qs = gen_passes(kernels)
    npt = sum(len(q["passes"]) for q in qs)
    arr = np.zeros((3, 128, npt * 126), np.float32)
    for cls in range(3):
        bands = gen_bands(n, cls, kernels)
        g = 0
        for qi, q in enumerate(qs):
            for pi in range(len(q["passes"])):
                arr[cls, :, g * 126:(g + 1) * 126] = bands[(qi, pi)]
                g += 1
    return np.ascontiguousarray(arr.astype(ml_dtypes.bfloat16))


# ---------------------------------------------------------------------------
# Bass module: uint8 in, 8 quarter-res uint8 planes out
def build_nc(H_, W_, kernels=None, num_devices=N_CORES,
             in_bufs=2, out_bufs=2, band_bufs=2, psum_bufs=8):
    import concourse.bacc as bacc
    import concourse.tile as tile
    import concourse.mybir as mybir

    F32 = mybir.dt.float32
    BF16 = mybir.dt.bfloat16
    U8 = mybir.dt.uint8

    n, wn = H_ // 2, W_ // 2
    NCH = min(512, wn)           # matmul moving free dim (one PSUM bank fp32)
    assert wn % NCH == 0
    nchunks = wn // NCH
    qs = gen_passes(kernels)
    gpi_of = {}
    g = 0
    for qi, q in enumerate(qs):
        for pi in range(len(q["passes"])):
            gpi_of[(qi, pi)] = g
            g += 1
    NPT = g
    plan = block_plan(n)

    nc = bacc.Bacc("TRN2", target_bir_lowering=False, debug=False,
                   enable_asserts=False, num_devices=num_devices)
    x = nc.dram_tensor("x", [H_, W_], U8, kind="ExternalInput").ap()
    bands_d = nc.dram_tensor("bands", [3, 128, NPT * 126], BF16,
                             kind="ExternalInput").ap()
    y = nc.dram_tensor("y", [len(qs), n, wn], U8, kind="ExternalOutput").ap()

    with ExitStack() as ctx:
        tc = ctx.enter_context(tile.TileContext(nc))
        in_pool = ctx.enter_context(tc.tile_pool(name="inp", bufs=in_bufs))
        band_pool = ctx.enter_context(tc.tile_pool(name="band", bufs=band_bufs))
        out_pool = ctx.enter_context(tc.tile_pool(name="outp", bufs=out_bufs))
        psum_pool = ctx.enter_context(tc.tile_pool(name="ps", bufs=psum_bufs,
                                                   space="PSUM"))
        band_tiles = {}

        def get_band_tile(cls):
            if cls not in band_tiles:
                bt = band_pool.tile([128, NPT * 126], BF16, tag="bands")
                nc.sync.dma_start(bt[:, :], bands_d[cls])
                band_tiles[cls] = bt
            return band_tiles[cls]

        for (base, out0, M, cls) in plan:
            bt = get_band_tile(cls)
            tin = {}
            for pr in (0, 1):
                t8 = in_pool.tile([128, W_], U8, tag=f"u{pr}")
                nc.sync.dma_start(t8[:, :],
                                  x[2 * base + pr: 2 * base + pr + 255: 2, :])
                t = in_pool.tile([128, W_ + 4], BF16, tag=f"t{pr}")
                nc.scalar.copy(t[:, 2:W_ + 2], t8[:, :])   # u8 -> bf16 cast
                # reflect-pad columns: tile col c <-> image col c-2
                nc.scalar.copy(t[:, 0:1], t[:, 4:5])
                nc.scalar.copy(t[:, 1:2], t[:, 3:4])
                nc.scalar.copy(t[:, W_ + 2:W_ + 3], t[:, W_:W_ + 1])
                nc.scalar.copy(t[:, W_ + 3:W_ + 4], t[:, W_ - 1:W_])
                tin[pr] = t
            A = [out_pool.tile([128, wn], U8, tag=f"A{qi}", name=f"A{qi}")
                 for qi in range(len(qs))]
            for qi, q in enumerate(qs):
                for c in range(nchunks):
                    ps = psum_pool.tile([128, NCH], F32, tag="ps")
                    for pi, p in enumerate(q["passes"]):
                        gp = gpi_of[(qi, pi)]
                        lhsT = bt[:, gp * 126: gp * 126 + 126]
                        c0 = 2 * p["dcol"] + p["pc"] + 2 + 2 * NCH * c
                        rhs = tin[p["pr"]][:, c0: c0 + 2 * NCH - 1: 2]
                        nc.tensor.matmul(ps[0:126, :], lhsT, rhs,
                                         start=(pi == 0),
                                         stop=(pi == len(q["passes"]) - 1))
                    # clip to [0,255] and round-to-nearest u8 store
                    nc.vector.tensor_scalar(
                        A[qi][0:126, NCH * c: NCH * (c + 1)], ps[0:126, :],
                        255.0, 0.0, mybir.AluOpType.min, mybir.AluOpType.max)
            for qi in range(len(qs)):
                nc.sync.dma_start(y[qi, out0: out0 + M, :], A[qi][0:M, :])
    nc.compile()
    return nc


# ---------------------------------------------------------------------------
# Dispatch: a slim replacement for run_bass_kernel_spmd's axon path that
# avoids per-call host concats, the host-zeros upload for output staging,
# and double-copied output gathers.
class _Runner:
    def __init__(self, h, w, kernels=None):
        import jax
        import jax.numpy as jnp
        from jax.sharding import Mesh, PartitionSpec, NamedSharding
        from jax.experimental.shard_map import shard_map
        import concourse.mybir as mybir
        from concourse import bass2jax

        bass2jax.install_neuronx_cc_hook()
        nc = build_nc(h, w, kernels)
        assert nc.dbg_addr is None
        self.nc = nc

        partition_name = (nc.partition_id_tensor.name
                          if nc.partition_id_tensor else None)
        in_names, out_names, out_avals = [], [], []
        for alloc in nc.m.functions[0].allocations:
            if not isinstance(alloc, mybir.MemoryLocationSet):
                continue
            name = alloc.memorylocations[0].name
            if alloc.kind == "ExternalInput":
                if name != partition_name:
                    in_names.append(name)
            elif alloc.kind == "ExternalOutput":
                assert alloc.tensor_shape is not None
                out_names.append(name)
                out_avals.append(jax.core.ShapedArray(
                    tuple(alloc.tensor_shape), mybir.dt.np(alloc.dtype)))
        assert in_names == ["x", "bands"] and out_names == ["y"], \
            (in_names, out_names)
        n_params, n_outs = len(in_names), len(out_avals)
        all_in = tuple(in_names + out_names +
                       ([partition_name] if partition_name else []))

        def _body(*args):
            operands = list(args)
            if partition_name is not None:
                operands.append(bass2jax.partition_id_tensor())
            outs = bass2jax._bass_exec_p.bind(
                *operands, out_avals=tuple(out_avals), in_names=all_in,
                out_names=tuple(out_names), lowering_input_output_aliases=(),
                sim_require_finite=True, sim_require_nnan=True, nc=nc)
            return tuple(outs)

        devices = jax.devices()[:N_CORES]
        assert len(devices) == N_CORES
        mesh = Mesh(np.asarray(devices), ("core",))
        self.sharding = NamedSharding(mesh, PartitionSpec("core"))
        in_specs = (PartitionSpec("core"),) * (n_params + n_outs)
        out_specs = (PartitionSpec("core"),) * n_outs
        donate = tuple(range(n_params, n_params + n_outs))
        self.fn = jax.jit(
            shard_map(_body, mesh=mesh, in_specs=in_specs,
                      out_specs=out_specs, check_rep=False),
            donate_argnums=donate, keep_unused=True)
        zshape = (N_CORES * out_avals[0].shape[0], *out_avals[0].shape[1:])
        zdtype = out_avals[0].dtype
        self.zeros_fn = jax.jit(lambda: jnp.zeros(zshape, zdtype),
                                out_shardings=self.sharding)
        self.band_cache = {}

    def bands_dev(self, key, h, kernels):
        import jax
        if key not in self.band_cache:
            bnp = build_bands_np(h // 2, kernels)
            tiled = np.ascontiguousarray(
                np.broadcast_to(bnp[None], (N_CORES,) + bnp.shape)
            ).reshape(N_CORES * bnp.shape[0], *bnp.shape[1:])
            self.band_cache[key] = jax.device_put(tiled, self.sharding)
        return self.band_cache[key]


_RUNNERS = {}
_LAST_RESULTS = None


def kernel(**inputs) -> np.ndarray:
    import jax

    bayer = np.asarray(inputs["bayer"], dtype=np.float32)
    b, c1, h, w = bayer.shape
    assert (b, c1, h, w) == (B, 1, H, W), bayer.shape

    kernels = None
    kkey = "default"
    if "k_g_at_rb" in inputs:
        kernels = {
            "g": np.asarray(inputs["k_g_at_rb"], np.float32).reshape(5, 5),
            "col": np.asarray(inputs["k_rb_at_g_col"], np.float32).reshape(5, 5),
            "row": np.asarray(inputs["k_rb_at_g_row"], np.float32).reshape(5, 5),
            "br": np.asarray(inputs["k_rb_at_br"], np.float32).reshape(5, 5),
        }
        kkey = b"".join(k.tobytes() for k in kernels.values())

    if (h, w) not in _RUNNERS:
        _RUNNERS[(h, w)] = _Runner(h, w, kernels)
    r = _RUNNERS[(h, w)]

    # quantize input to u8 (values bayer*255, round-to-nearest; bayer is in
    # [0,1) so the +0.5 truncation cast == rint and cannot overflow)
    flat = bayer.reshape(B * h, w)
    xq = (flat * np.float32(255.0) + np.float32(0.5)).astype(np.uint8)

    x_dev = jax.device_put(xq, r.sharding)
    bands_dev = r.bands_dev(kkey, h, kernels)
    zeros = r.zeros_fn()
    (out,) = r.fn(x_dev, bands_dev, zeros)

    # overlap D2H of the 8 per-core shards with host-side assembly
    shards = sorted(out.addressable_shards,
                    key=lambda s: (s.index[0].start or 0))
    for s in shards:
        s.data.copy_to_host_async()
    final = np.empty((B, 3, h, w), np.float32)
    lut = np.arange(256, dtype=np.float32) * np.float32(1.0 / 255.0)
    for s in shards:
        bidx = (s.index[0].start or 0) // len(CONV_OUTPUTS)
        planes = np.asarray(s.data)          # [8, h/2, w/2] u8
        fb = final[bidx]
        for k, (ch, di, dj, _) in enumerate(CONV_OUTPUTS):
            fb[ch, di::2, dj::2] = lut[planes[k]]
        xb = bayer[bidx, 0]
        for (ch, di, dj) in PASSTHROUGH_OUTPUTS:
            np.clip(xb[di::2, dj::2], 0.0, 1.0, out=fb[ch, di::2, dj::2])
    return final


if __name__ == "__main__":
    qs = gen_passes()
    for q in qs:
        print(q["ch"], q["di0"], q["dj0"], "passes:", len(q["passes"]))
    print("total passes:", sum(len(q["passes"]) for q in qs))
    print("plan n=1024:", block_plan(1024))
